# revision 3
# baseline (speedup 1.0000x reference)
"""EvolveGCN (2-layer GRCU + TopK + matrix-GRU) on 8 Trainium2 NeuronCores.

Strategy: row-shard A over cores (each core owns 512 rows of the N=4096
nodes); X/weights replicated. Per timestep, each core computes the full
(replicated) TopK+GRU weight evolution and the Z=emb@W product, then its
512-row shard of H=relu(A@Z) (as H.T), which is AllGathered so every core
holds the full H.T for the next layer. Everything is fp32 (f32r's ~1e-4
relative error flips TopK orderings; the instance has adjacent score gaps
down to 3e-7 of std).

kernel(**inputs) takes FULL inputs (as produced by the problem's
setup_inputs) and returns the FULL (4096, 3) output.
"""
import os
os.environ.setdefault("JAX_PLATFORMS", "cpu")
import numpy as np

T, N, F0, F1, F2, C = 6, 4096, 256, 256, 128, 3
NCORES = 8
SH = N // NCORES  # 512 rows per core

_compiled = {}


def _emit_topk(nc, sb, ps, dram, s128, K, emb_halves, uid):
    """s128 (128, 32) fp32 (s[p*32+j] at [p,j]) -> (ht halves [(128, K)]*2, ) exact top-K,
    columns ordered by descending score; ht[:, j] = emb[idx_j]*tanh(val_j).
    Precomputed tiles (iota etc) are cached on nc._topk_cache."""
    import concourse.mybir as mybir
    f32 = mybir.dt.float32
    i16 = mybir.dt.int16
    u32 = mybir.dt.uint32
    i32 = mybir.dt.int32
    AOT = mybir.AluOpType
    ACT = mybir.ActivationFunctionType

    cache = getattr(nc, "_topk_cache", None)
    if cache is None:
        cache = nc._topk_cache = {}
    if "p32f" not in cache:
        p32 = sb.tile([128, 1], i32, tag="tk_p32", name="tk_p32")
        nc.gpsimd.iota(p32, pattern=[[0, 1]], base=0, channel_multiplier=32)
        p32f = sb.tile([128, 1], f32, tag="tk_p32f", name="tk_p32f")
        nc.vector.tensor_copy(p32f, p32)
        cache["p32f"] = p32f
        ii = sb.tile([128, 256], i32, tag="tk_ii", name="tk_ii")
        nc.gpsimd.iota(ii, pattern=[[1, 256]], base=0, channel_multiplier=0)
        iif = sb.tile([128, 256], f32, tag="tk_iif", name="tk_iif")
        nc.vector.tensor_copy(iif, ii)
        cache["iif"] = iif
    p32f = cache["p32f"]
    iif = cache["iif"]

    V8 = sb.tile([128, 8], f32, tag="tk_V8", name=f"tk_V8_{uid}")
    I8 = sb.tile([128, 8], u32, tag="tk_I8", name=f"tk_I8_{uid}")
    nc.vector.max(out=V8, in_=s128)
    nc.vector.max_index(out=I8, in_max=V8, in_values=s128)
    gI8 = sb.tile([128, 8], f32, tag="tk_gI8", name=f"tk_gI8_{uid}")
    nc.vector.tensor_copy(gI8, I8)
    nc.vector.tensor_scalar_add(gI8, gI8, p32f)
    tanh8 = sb.tile([128, 8], f32, tag="tk_tanh8", name=f"tk_tanh8_{uid}")
    nc.scalar.activation(tanh8, V8, ACT.Tanh)

    cand_row = sb.tile([1, 1024], f32, tag="tk_crow", name=f"tk_crow_{uid}")
    nc.sync.dma_start(out=cand_row, in_=V8)
    cand_bc = sb.tile([128, 1024], f32, tag="tk_cbc", name=f"tk_cbc_{uid}")
    nc.gpsimd.partition_broadcast(cand_bc, cand_row)

    rank8 = sb.tile([128, 8], f32, tag="tk_rank8", name=f"tk_rank8_{uid}")
    scratch = sb.tile([128, 1024], f32, tag="tk_scr", name=f"tk_scr_{uid}")
    for j in range(8):
        nc.vector.tensor_scalar(
            out=scratch, in0=cand_bc, scalar1=V8[:, j:j + 1], scalar2=None,
            op0=AOT.is_gt, op1=AOT.add, accum_out=rank8[:, j:j + 1])

    GT = sb.tile([128, 16], f32, tag="tk_GT", name=f"tk_GT_{uid}")
    nc.vector.tensor_copy(GT[:, 0:8], gI8)
    nc.vector.tensor_copy(GT[:, 8:16], tanh8)
    p_gi = ps.tile([1, 512], f32, tag="s", name=f"tk_pgi_{uid}")
    p_th = ps.tile([1, 512], f32, tag="s", name=f"tk_pth_{uid}")
    for j in range(8):
        IND = sb.tile([128, 256], f32, tag="tk_IND", name=f"tk_IND_{uid}_{j}")
        nc.vector.tensor_scalar(
            out=IND[:, :K], in0=iif[:, :K], scalar1=rank8[:, j:j + 1],
            scalar2=None, op0=AOT.is_equal)
        nc.tensor.matmul(out=p_gi[:, :K], lhsT=GT[:, j:j + 1], rhs=IND[:, :K],
                         start=(j == 0), stop=(j == 7))
        nc.tensor.matmul(out=p_th[:, :K], lhsT=GT[:, 8 + j:8 + j + 1],
                         rhs=IND[:, :K], start=(j == 0), stop=(j == 7))
    ord_gi = sb.tile([1, 256], f32, tag="tk_ogi", name=f"tk_ogi_{uid}")
    ord_th = sb.tile([1, 256], f32, tag="tk_oth", name=f"tk_oth_{uid}")
    nc.vector.tensor_copy(ord_gi[:, :K], p_gi[:, :K])
    nc.vector.tensor_copy(ord_th[:, :K], p_th[:, :K])

    gi16 = sb.tile([1, 256], i16, tag="tk_gi16", name=f"tk_gi16_{uid}")
    nc.vector.tensor_copy(gi16[:, :K], ord_gi[:, :K])
    idx_dram = dram.tile([1, 256], i16, tag="tk_idxd", name=f"tk_idxd_{uid}")
    nc.sync.dma_start(out=idx_dram[:, :K], in_=gi16[:, :K])
    idx_w = sb.tile([128, 16], i16, tag="tk_idxw", name=f"tk_idxw_{uid}")
    for gg in range(8):
        nc.sync.dma_start(
            out=idx_w[gg * 16:(gg + 1) * 16, :K // 16],
            in_=idx_dram[:, :K].rearrange("o (f p) -> (o p) f", p=16))

    tanh_bc = sb.tile([128, 256], f32, tag="tk_thbc", name=f"tk_thbc_{uid}")
    nc.gpsimd.partition_broadcast(tanh_bc[:, :K], ord_th[:, :K])
    ht = []
    for h in range(2):
        gath = sb.tile([128, 256], f32, tag=f"tk_gath{h}",
                       name=f"tk_gath{h}_{uid}")
        nc.gpsimd.ap_gather(gath[:, :K], emb_halves[h], idx_w[:, :K // 16],
                            channels=128, num_elems=N, d=1, num_idxs=K)
        ht_h = sb.tile([128, 256], f32, tag=f"tk_ht{h}", name=f"tk_ht{h}_{uid}")
        nc.vector.tensor_mul(ht_h[:, :K], gath[:, :K], tanh_bc[:, :K])
        ht.append(ht_h)
    return ht


def _emit_gru(nc, sb, ps, W_st, ht, gates, biases, fout, uid):
    """Matrix GRU step, in place on state tiles W_st ([(128, fout)]*2).
    gates: dict of 6 transposed weights, each [(128, 256)]*2 (lhsT layout).
    biases: dict of 3 biases, each [(128, fout)]*2."""
    import concourse.mybir as mybir
    f32 = mybir.dt.float32
    ACT = mybir.ActivationFunctionType
    WzT, UzT, WrT, UrT, WhT, UhT = (gates[k] for k in
                                    ("WzT", "UzT", "WrT", "UrT", "WhT", "UhT"))
    bz, br, bh = (biases[k] for k in ("bz", "br", "bh"))

    def gate_mm(w_t, u_t, rhs_w, rhs_u, m):
        p = ps.tile([128, 256], f32, tag="gru", name=f"gru_p_{uid}_{m}")
        first = True
        for kt in range(2):
            nc.tensor.matmul(out=p[:, :fout], lhsT=w_t[kt][:, m * 128:(m + 1) * 128],
                             rhs=rhs_w[kt][:, :fout], start=first, stop=False)
            first = False
        for kt in range(2):
            last = kt == 1
            nc.tensor.matmul(out=p[:, :fout], lhsT=u_t[kt][:, m * 128:(m + 1) * 128],
                             rhs=rhs_u[kt][:, :fout], start=False, stop=last)
        return p

    z, r = [], []
    for m in range(2):
        pz = gate_mm(WzT, UzT, ht, W_st, m)
        z_m = sb.tile([128, 256], f32, tag=f"gru_z{m}", name=f"gru_z{m}_{uid}")
        nc.vector.tensor_add(z_m[:, :fout], pz[:, :fout], bz[m])
        nc.scalar.activation(z_m[:, :fout], z_m[:, :fout], ACT.Sigmoid)
        z.append(z_m)
    for m in range(2):
        pr = gate_mm(WrT, UrT, ht, W_st, m)
        r_m = sb.tile([128, 256], f32, tag=f"gru_r{m}", name=f"gru_r{m}_{uid}")
        nc.vector.tensor_add(r_m[:, :fout], pr[:, :fout], br[m])
        nc.scalar.activation(r_m[:, :fout], r_m[:, :fout], ACT.Sigmoid)
        r.append(r_m)
    rw = []
    for m in range(2):
        rw_m = sb.tile([128, 256], f32, tag=f"gru_rw{m}", name=f"gru_rw{m}_{uid}")
        nc.vector.tensor_mul(rw_m[:, :fout], r[m][:, :fout], W_st[m][:, :fout])
        rw.append(rw_m)
    for m in range(2):
        ph = gate_mm(WhT, UhT, ht, rw, m)
        hc_m = sb.tile([128, 256], f32, tag=f"gru_hc{m}", name=f"gru_hc{m}_{uid}")
        nc.vector.tensor_add(hc_m[:, :fout], ph[:, :fout], bh[m])
        nc.scalar.activation(hc_m[:, :fout], hc_m[:, :fout], ACT.Tanh)
        # Wn = W + z*(hc - W)
        nc.vector.tensor_sub(hc_m[:, :fout], hc_m[:, :fout], W_st[m][:, :fout])
        nc.vector.tensor_mul(hc_m[:, :fout], hc_m[:, :fout], z[m][:, :fout])
        nc.vector.tensor_add(W_st[m][:, :fout], W_st[m][:, :fout], hc_m[:, :fout])


def build_kernel(reps=1):
    """Build the SPMD program. Returns finalized Bacc object."""
    import concourse.bacc as bacc
    import concourse.mybir as mybir
    from concourse.tile import TileContext
    f32 = mybir.dt.float32
    AOT = mybir.AluOpType
    ACT = mybir.ActivationFunctionType

    nc = bacc.Bacc("TRN2", target_bir_lowering=False, debug=False,
                   num_devices=NCORES)
    at_d = nc.declare_dram_parameter("AT", [T, N, SH], f32, isOutput=False)
    xt_d = nc.declare_dram_parameter("XT", [T, F0, N], f32, isOutput=False)
    par = {}
    for li, (fin, fout) in enumerate(((F0, F1), (F1, F2))):
        for g in ("WzT", "UzT", "WrT", "UrT", "WhT", "UhT"):
            par[f"{g}{li}"] = nc.declare_dram_parameter(
                f"{g}{li}", [fin, fin], f32, isOutput=False)
        for b in ("bz", "br", "bh"):
            par[f"{b}{li}"] = nc.declare_dram_parameter(
                f"{b}{li}", [fin, fout], f32, isOutput=False)
        par[f"Wi{li}"] = nc.declare_dram_parameter(
            f"Wi{li}", [fin, fout], f32, isOutput=False)
        par[f"u{li}"] = nc.declare_dram_parameter(
            f"u{li}", [fin, 1], f32, isOutput=False)
    clfw_d = nc.declare_dram_parameter("clfW", [F2, C], f32, isOutput=False)
    clfb_d = nc.declare_dram_parameter("clfb", [1, C], f32, isOutput=False)
    out_d = nc.declare_dram_parameter("out", [SH, C], f32, isOutput=True)

    with TileContext(nc) as tc:
        with (
            tc.tile_pool(name="stat", bufs=1) as st,
            tc.tile_pool(name="sb", bufs=1) as sb,
            tc.tile_pool(name="xt", bufs=1) as xtp,
            tc.tile_pool(name="big", bufs=1) as big,
            tc.tile_pool(name="ap", bufs=8) as app,
            tc.tile_pool(name="ps", bufs=2, space="PSUM") as ps,
            tc.tile_pool(name="dram", bufs=2, space="DRAM") as dram,
        ):
            # ---- static weights ----
            def load_pair(name, fout_cols):
                tiles = []
                for kt in range(2):
                    tl = st.tile([128, fout_cols], f32, tag=f"st_{name}_{kt}",
                                 name=f"st_{name}_{kt}")
                    nc.sync.dma_start(
                        out=tl, in_=par[name][kt * 128:(kt + 1) * 128, :])
                    tiles.append(tl)
                return tiles

            gates, biases, us = [], [], []
            for li, (fin, fout) in enumerate(((F0, F1), (F1, F2))):
                gates.append({g: load_pair(f"{g}{li}", fin)
                              for g in ("WzT", "UzT", "WrT", "UrT", "WhT", "UhT")})
                biases.append({b: load_pair(f"{b}{li}", fout)
                               for b in ("bz", "br", "bh")})
                u_t = []
                for kt in range(2):
                    tl = st.tile([128, 1], f32, tag=f"st_u{li}_{kt}",
                                 name=f"st_u{li}_{kt}")
                    nc.sync.dma_start(
                        out=tl, in_=par[f"u{li}"][kt * 128:(kt + 1) * 128, :])
                    u_t.append(tl)
                us.append(u_t)
            clfw = st.tile([128, C], f32, tag="st_clfw", name="st_clfw")
            nc.sync.dma_start(out=clfw, in_=clfw_d[:, :])
            clfb_row = st.tile([1, C], f32, tag="st_clfb", name="st_clfb")
            nc.sync.dma_start(out=clfb_row, in_=clfb_d[:, :])
            clfb_bc = st.tile([128, C], f32, tag="st_clfbb", name="st_clfbb")
            nc.gpsimd.partition_broadcast(clfb_bc, clfb_row)

            W_st = []  # persistent GRU states
            for li, fout in enumerate((F1, F2)):
                tiles = []
                for kt in range(2):
                    tl = st.tile([128, 256], f32, tag=f"st_W{li}_{kt}",
                                 name=f"st_W{li}_{kt}")
                    tiles.append(tl)
                W_st.append(tiles)

            for rep in range(reps):
                # (re)init GRU states
                for li in range(2):
                    for kt in range(2):
                        fout = (F1, F2)[li]
                        nc.sync.dma_start(
                            out=W_st[li][kt][:, :fout],
                            in_=par[f"Wi{li}"][kt * 128:(kt + 1) * 128, :])

                for t in range(T):
                    uid = f"r{rep}t{t}"
                    # ---- load XT_t ----
                    xt = []
                    for h in range(2):
                        tl = xtp.tile([128, N], f32, tag=f"xt{h}",
                                      name=f"xt{h}_{uid}")
                        nc.sync.dma_start(
                            out=tl, in_=xt_d[t, h * 128:(h + 1) * 128, :])
                        xt.append(tl)

                    # ---- layer 0 scores (straight into (128,32) layout) ----
                    s128_0 = sb.tile([128, 32], f32, tag="s128t",
                                     name=f"s128_0_{uid}")
                    for j in range(8):
                        cs = slice(j * 512, (j + 1) * 512)
                        p = ps.tile([1, 512], f32, tag="s", name=f"ps_s0_{uid}_{j}")
                        for kt in range(2):
                            nc.tensor.matmul(out=p, lhsT=us[0][kt],
                                             rhs=xt[kt][:, cs],
                                             start=(kt == 0), stop=(kt == 1))
                        sc = sb.tile([1, 512], f32, tag=f"s_ch{j % 2}",
                                     name=f"s_ch0_{uid}_{j}")
                        nc.vector.tensor_copy(sc, p)
                        nc.sync.dma_start(out=s128_0[j * 16:(j + 1) * 16, :], in_=sc)

                    ht0 = _emit_topk(nc, sb, ps, dram, s128_0, F1, xt, f"l0{uid}")
                    _emit_gru(nc, sb, ps, W_st[0], ht0, gates[0], biases[0],
                              F1, f"g0{uid}")

                    # ---- Z0 = X @ Wn0 (natural layout) ----
                    z0sb = big.tile([128, 32 * 256], f32, tag="z0sb",
                                    name=f"z0sb_{uid}")
                    for n in range(32):
                        pz = ps.tile([128, 256], f32, tag="z", name=f"ps_z0_{uid}_{n}")
                        for kt in range(2):
                            nc.tensor.matmul(
                                out=pz, lhsT=xt[kt][:, n * 128:(n + 1) * 128],
                                rhs=W_st[0][kt], start=(kt == 0), stop=(kt == 1))
                        nc.vector.tensor_copy(
                            z0sb[:, n * 256:(n + 1) * 256], pz)

                    # ---- H1T shard = relu(A @ Z0).T ----
                    h1sh = []
                    for h in range(2):
                        ph = ps.tile([128, 512], f32, tag="h", name=f"ps_h1_{uid}_{h}")
                        for n in range(32):
                            a_t = app.tile([128, 512], f32, tag="a_t",
                                           name=f"a1_{uid}_{h}_{n}")
                            nc.sync.dma_start(
                                out=a_t, in_=at_d[t, n * 128:(n + 1) * 128, :])
                            nc.tensor.matmul(
                                out=ph,
                                lhsT=z0sb[:, n * 256 + h * 128:n * 256 + (h + 1) * 128],
                                rhs=a_t, start=(n == 0), stop=(n == 31))
                        sh_h = sb.tile([128, 512], f32, tag=f"h1sh{h}",
                                       name=f"h1sh{h}_{uid}")
                        nc.vector.tensor_scalar_max(sh_h, ph, 0.0)
                        h1sh.append(sh_h)

                    # ---- AllGather H1T ----
                    ag_in = dram.tile([F1, SH], f32, tag="ag_in",
                                      name=f"ag_in_{uid}")
                    for h in range(2):
                        nc.sync.dma_start(
                            out=ag_in[h * 128:(h + 1) * 128, :], in_=h1sh[h])
                    ag_out = dram.tile([NCORES, F1, SH], f32, tag="ag_out",
                                       name=f"ag_out_{uid}")
                    nc.gpsimd.collective_compute(
                        "AllGather", AOT.bypass,
                        replica_groups=[list(range(NCORES))],
                        ins=[ag_in.opt()], outs=[ag_out.opt()])
                    h1f = []
                    for h in range(2):
                        tl = big.tile([128, N], f32, tag=f"h1f{h}",
                                      name=f"h1f{h}_{uid}")
                        nc.sync.dma_start(
                            out=tl.rearrange("p (c j) -> p c j", c=NCORES),
                            in_=ag_out[:, h * 128:(h + 1) * 128, :]
                            .rearrange("c p j -> p c j"))
                        h1f.append(tl)

                    # ---- layer 1 scores + state update ----
                    s128_1 = sb.tile([128, 32], f32, tag="s128t",
                                     name=f"s128_1_{uid}")
                    for j in range(8):
                        cs = slice(j * 512, (j + 1) * 512)
                        p = ps.tile([1, 512], f32, tag="s", name=f"ps_s1_{uid}_{j}")
                        for kt in range(2):
                            nc.tensor.matmul(out=p, lhsT=us[1][kt],
                                             rhs=h1f[kt][:, cs],
                                             start=(kt == 0), stop=(kt == 1))
                        sc = sb.tile([1, 512], f32, tag=f"s_ch{j % 2}",
                                     name=f"s_ch1_{uid}_{j}")
                        nc.vector.tensor_copy(sc, p)
                        nc.sync.dma_start(out=s128_1[j * 16:(j + 1) * 16, :], in_=sc)

                    ht1 = _emit_topk(nc, sb, ps, dram, s128_1, F2, h1f, f"l1{uid}")
                    _emit_gru(nc, sb, ps, W_st[1], ht1, gates[1], biases[1],
                              F2, f"g1{uid}")

                    if t == T - 1:
                        # ---- Z1 = H1 @ Wn1; H2T = relu(A @ Z1).T; out ----
                        z1sb = big.tile([128, 32 * 128], f32, tag="z1sb",
                                        name=f"z1sb_{uid}")
                        for n in range(32):
                            pz = ps.tile([128, 256], f32, tag="z",
                                         name=f"ps_z1_{uid}_{n}")
                            for kt in range(2):
                                nc.tensor.matmul(
                                    out=pz[:, :F2],
                                    lhsT=h1f[kt][:, n * 128:(n + 1) * 128],
                                    rhs=W_st[1][kt][:, :F2],
                                    start=(kt == 0), stop=(kt == 1))
                            nc.vector.tensor_copy(
                                z1sb[:, n * 128:(n + 1) * 128], pz[:, :F2])
                        ph2 = ps.tile([128, 512], f32, tag="h", name=f"ps_h2_{uid}")
                        for n in range(32):
                            a_t = app.tile([128, 512], f32, tag="a_t",
                                           name=f"a2_{uid}_{n}")
                            nc.sync.dma_start(
                                out=a_t, in_=at_d[t, n * 128:(n + 1) * 128, :])
                            nc.tensor.matmul(
                                out=ph2, lhsT=z1sb[:, n * 128:(n + 1) * 128],
                                rhs=a_t, start=(n == 0), stop=(n == 31))
                        h2t = sb.tile([128, 512], f32, tag="h2t",
                                      name=f"h2t_{uid}")
                        nc.vector.tensor_scalar_max(h2t, ph2, 0.0)
                        for m in range(4):
                            po = ps.tile([128, 256], f32, tag="z",
                                         name=f"ps_o_{uid}_{m}")
                            nc.tensor.matmul(
                                out=po[:, :C],
                                lhsT=h2t[:, m * 128:(m + 1) * 128],
                                rhs=clfw, start=True, stop=True)
                            o_m = sb.tile([128, C], f32, tag="o_m",
                                          name=f"o_m_{uid}_{m}")
                            nc.vector.tensor_add(o_m, po[:, :C], clfb_bc)
                            nc.sync.dma_start(
                                out=out_d[m * 128:(m + 1) * 128, :], in_=o_m)

    nc.finalize()
    return nc


def _prep_inputs(inputs):
    """Host-side shard/transpose prep. Returns per-core in_maps."""
    A = np.asarray(inputs["A"], np.float32)
    X = np.asarray(inputs["X"], np.float32)
    AT = np.ascontiguousarray(A.transpose(0, 2, 1))  # (T, N, N): AT[t] = A[t].T
    XT = np.ascontiguousarray(X.transpose(0, 2, 1))  # (T, F0, N)
    common = {"XT": XT}
    for li in range(2):
        sfx = f"_{li}"
        for g in ("Wz", "Uz", "Wr", "Ur", "Wh", "Uh"):
            common[f"{g}T{li}"] = np.ascontiguousarray(
                np.asarray(inputs[g + sfx], np.float32).T)
        for b in ("bz", "br", "bh"):
            common[f"{b}{li}"] = np.asarray(inputs[b + sfx], np.float32)
        common[f"Wi{li}"] = np.asarray(inputs["W_init" + sfx], np.float32)
        scorer = np.asarray(inputs["scorer" + sfx], np.float64)
        common[f"u{li}"] = (scorer / np.linalg.norm(scorer)).astype(np.float32)
    common["clfW"] = np.asarray(inputs["clf_W"], np.float32)
    common["clfb"] = np.asarray(inputs["clf_b"], np.float32).reshape(1, C)
    in_maps = []
    for c in range(NCORES):
        m = dict(common)
        m["AT"] = np.ascontiguousarray(AT[:, :, c * SH:(c + 1) * SH])
        in_maps.append(m)
    return in_maps


def kernel(**inputs):
    from concourse.bass_utils import run_bass_kernel_spmd
    if "nc" not in _compiled:
        _compiled["nc"] = build_kernel(reps=1)
    nc = _compiled["nc"]
    in_maps = _prep_inputs(inputs)
    res = run_bass_kernel_spmd(nc, in_maps, list(range(NCORES)))
    out = np.concatenate([res.results[c]["out"] for c in range(NCORES)], axis=0)
    return out.astype(np.float32)


if __name__ == "__main__":
    import reference as R
    inputs = {k: np.asarray(v) for k, v in R.setup_inputs().items()}
    got = kernel(**inputs)
    exp = np.asarray(R.reference(**inputs))
    err = np.abs(got - exp).max() / np.abs(exp).max()
    print("rel err vs jax fp32 reference:", err)
